# revision 9
# baseline (speedup 1.0000x reference)
"""GCN-3 Trainium2 kernel v2 — 3 SpMM launches + head, 8 cores.

Structure vs baseline:
- B1 (dest=items, gathers ue f32) -> A12 (dest=users, dual-layer gather of
  [ie|g1_i] bf16, computes g1_u AND g2_u + gcn_u in one pass) -> B2
  (dest=items, gathers g1_u bf16-padded, computes g2_i + gcn_i) -> head.
- 64-dest half-cells: S matrices [slot,64]; two halves packed vertically in
  PSUM via column-tiled matmuls (tile_position (0,0)/(0,64)).
- PSUM-resident accumulation over col-ranges (no SBUF acc, no per-window
  evictions); residual/gcn combine injected via static identity matmuls,
  relu evictions on ACT straight from PSUM.
- val-scale on ACT (fused with f32->bf16 cast where applicable).
"""
import os
import sys

sys.path.insert(0, '/opt/trn_rl_repo')
os.environ.setdefault("MYCRO_LOCAL_CACHE", "1")
os.environ.setdefault("NEURON_RT_RESET_CORES", "1")

import numpy as np
import ml_dtypes
import concourse.bacc as bacc
import concourse.bass as bass
import concourse.mybir as mybir
from concourse.tile import TileContext
from concourse.bass_utils import run_bass_kernel_spmd

F32 = mybir.dt.float32
BF16 = mybir.dt.bfloat16
I16 = mybir.dt.int16
AL = mybir.AluOpType
AF = mybir.ActivationFunctionType
BF = ml_dtypes.bfloat16

U = 359347
I = 292589
D = 64
B = 32768
LAM = 0.001

WIN = 32768
TSLOT = 2048
NCORES = 8

_EXEC_NS = {"total": 0}


# ----------------------------------------------------------------- planner --

def _side_mapping(n_orig):
    nd = int(np.ceil(n_orig / NCORES))
    C = int(np.ceil(nd / 128))
    rows_per_core = 128 * C
    n_pad = NCORES * rows_per_core
    ids = np.arange(n_orig)
    core = np.minimum(ids // nd, NCORES - 1)
    local = ids - core * nd
    storage = core * rows_per_core + (local % 128) * C + (local // 128)
    return dict(nd=nd, C=C, rows_per_core=rows_per_core, n_pad=n_pad,
                core=core.astype(np.int32), local=local.astype(np.int64),
                storage=storage.astype(np.int64))


def _plan_v2(dest_core, dest_local, src_store, vals, C, ns_pad, rcols):
    """Range/half-cell plan. Cells keyed (range, window, col-in-range, half).
    Window blocks padded to 128 slots; ranges padded to TSLOT."""
    nw = int(np.ceil(ns_pad / WIN))
    nr = int(np.ceil(C / rcols))
    C_pad = nr * rcols

    col = (dest_local // 128).astype(np.int64)
    hposv = (dest_local % 128).astype(np.float32)
    w = (src_store // WIN).astype(np.int64)
    idx16 = (src_store % WIN).astype(np.int16)

    r = col // rcols
    cr = col % rcols
    ncell = nr * nw * rcols
    cid = (r * nw + w) * rcols + cr
    keys = dest_core.astype(np.int64) * ncell + cid
    cnt = np.bincount(keys, minlength=NCORES * ncell).reshape(NCORES, ncell)
    npad = cnt.max(axis=0).astype(np.int64)
    # force >=1 slot in the w==0 cell of every (r, cr) so every psum slice
    # has at least one S-matmul (injections can then use start=False).
    w0cells = (np.arange(nr)[:, None] * nw * rcols
               + np.arange(rcols)[None, :]).ravel()
    npad[w0cells] = np.maximum(npad[w0cells], 1)

    # offsets: iterate ranges, windows; pad window block to 128, range to TSLOT
    off = np.zeros(ncell, np.int64)
    win_of = []            # per 128-slot block, the window id
    base = 0
    range_tile_lo = []
    for rr in range(nr):
        r_start = base
        for ww in range(nw):
            c0 = (rr * nw + ww) * rcols
            cells = np.arange(c0, c0 + rcols)
            sizes = npad[cells]
            co = np.concatenate([[0], np.cumsum(sizes)])
            off[cells] = base + co[:-1]
            blk = int(co[-1])
            blk_pad = (blk + 127) // 128 * 128
            win_of.extend([ww] * (blk_pad // 128))
            base += blk_pad
        # pad range to TSLOT
        rng = base - r_start
        rng_pad = (rng + TSLOT - 1) // TSLOT * TSLOT
        lastw = win_of[-1] if win_of else 0
        win_of.extend([lastw] * ((rng_pad - rng) // 128))
        base += rng_pad - rng
        range_tile_lo.append((r_start // TSLOT, base // TSLOT))
    total = int(base)
    T = total // TSLOT
    win_of = np.asarray(win_of, np.int64)  # [total//128]

    # slot assignment
    order = np.lexsort((dest_local, keys))
    sk = keys[order]
    first = np.r_[True, sk[1:] != sk[:-1]]
    grp_start = np.flatnonzero(first)
    grp_id = np.cumsum(first) - 1
    rank = np.arange(len(sk)) - grp_start[grp_id]
    e_core = dest_core[order]
    slot = off[cid[order]] + rank
    slot_idx = np.zeros((NCORES, total), np.int16)
    slot_val = np.zeros((NCORES, total), np.float32)
    slot_pos = np.full((NCORES, total), -10000.0, np.float32)
    slot_idx[e_core, slot] = idx16[order]
    slot_val[e_core, slot] = vals[order]
    slot_pos[e_core, slot] = hposv[order]

    # pairs per tile: pieces of cells within 128-slot groups
    cell_lo = off
    cell_hi = off + npad
    # order cells by lo for sweep
    live_cells = np.flatnonzero(npad > 0)
    lo_sorted = live_cells[np.argsort(cell_lo[live_cells])]
    pairs_all = [[] for _ in range(T)]
    seen_first = np.zeros(ncell, bool)
    # decode cid -> (r, w, cr)
    def _dec(c):
        crr = c % rcols
        c //= rcols
        ww = c % nw
        rr = c // nw
        return rr, ww, crr

    started = set()  # (r, cr) psum slices already started
    np_counts = np.zeros(T, np.int64)
    for c_ in lo_sorted:
        lo = int(cell_lo[c_]); hi = int(cell_hi[c_])
        rr, ww, crr = _dec(int(c_))
        key = (rr, crr)
        for g in range(lo // 128, (hi - 1) // 128 + 1):
            glo = max(lo, g * 128)
            ghi = min(hi, (g + 1) * 128)
            t = g // (TSLOT // 128)
            gg = g % (TSLOT // 128)
            st = key not in started
            started.add(key)
            pairs_all[t].append((int(np_counts[t]), gg, crr, st,
                                 glo - g * 128, ghi - g * 128, g))
            np_counts[t] += 1
    NPMAX = int((np_counts.max() + 7) // 8 * 8)

    posq = np.full((NCORES, T, 128, NPMAX), -10000.0, np.float32)
    for t in range(T):
        for (np_i, gg, crr, st, a, b, g) in pairs_all[t]:
            ks = np.arange(a, b)
            sl = np.arange(g * 128 + a, g * 128 + b)
            posq[:, t, ks, np_i] = slot_pos[:, sl]

    # gather segments per tile: runs of equal window, <=1024, 128-aligned
    segs_all = []
    for t in range(T):
        blocks = win_of[t * 16:(t + 1) * 16]
        segs = []
        s0 = 0
        for b_ in range(1, 17):
            if b_ == 16 or blocks[b_] != blocks[s0]:
                ln = (b_ - s0) * 128
                o = s0 * 128
                while ln > 0:
                    take = min(ln, 1024)
                    segs.append((o, take, int(blocks[s0])))
                    o += take
                    ln -= take
                s0 = b_
        segs_all.append(segs)

    idx_t = slot_idx.reshape(NCORES, T, TSLOT // 16, 16).transpose(0, 1, 3, 2)
    idx_t = np.ascontiguousarray(np.tile(idx_t, (1, 1, 8, 1)))
    val_t = np.ascontiguousarray(
        slot_val.reshape(NCORES, T, TSLOT // 128, 128).transpose(0, 1, 3, 2))
    return dict(T=T, nw=nw, nr=nr, C_pad=C_pad, rcols=rcols, NPMAX=NPMAX,
                pairs=pairs_all, segs=segs_all, idx=idx_t, val=val_t,
                pos=np.ascontiguousarray(posq).astype(BF),
                range_tiles=range_tile_lo,
                win_lo=[wi * WIN for wi in range(nw)],
                win_hi=[min((wi + 1) * WIN, ns_pad) for wi in range(nw)])


# ----------------------------------------------------------------- builder --

def _build_side_v2(plan, ns_pad, mode, w1=0.0, w2=0.0):
    """mode: 's1' (f32 src, g1 out), 'dual' (bf16x128 src, g1+gcn+ssq out),
    's2' (bf16x128 padded src, gcn+ssq out)."""
    T, NP_, rcols = plan['T'], plan['NPMAX'], plan['rcols']
    C_pad = plan['C_pad']
    dual = mode == 'dual'
    WSRC = 128 if mode != 's1' else 64     # src row elems
    WR = 128 if dual else 64               # rhs width per matmul
    cpb = 4 if dual else 8                 # cols per psum bank
    n12 = rcols // cpb                     # spmm psum banks per range

    nc = bacc.Bacc(num_swdge_queues=4)
    src = nc.dram_tensor("src", [ns_pad, WSRC], F32 if mode == 's1' else BF16,
                         kind="ExternalInput")
    idx_d = nc.dram_tensor("idx", [T, 128, TSLOT // 16], I16,
                           kind="ExternalInput")
    pos_d = nc.dram_tensor("pos", [T, 128, NP_], BF16, kind="ExternalInput")
    val_d = nc.dram_tensor("val", [T, 128, TSLOT // 128], F32,
                           kind="ExternalInput")
    rd_d = nc.dram_tensor("rd", [128, C_pad, D], BF16, kind="ExternalInput")
    if dual:
        w0ue_d = nc.dram_tensor("w0ue", [128, C_pad, D], BF16,
                                kind="ExternalInput")
        dvec_d = nc.dram_tensor("dvec", [128, C_pad], F32,
                                kind="ExternalInput")
    if mode == 's2':
        wcomb_d = nc.dram_tensor("wcomb", [128, C_pad, D], BF16,
                                 kind="ExternalInput")
    if mode in ('s1', 'dual'):
        g1_out = nc.dram_tensor("g1_out", [128, C_pad, D], BF16,
                                kind="ExternalOutput")
    if mode in ('dual', 's2'):
        gcn_out = nc.dram_tensor("gcn_out", [128, C_pad, D], BF16,
                                 kind="ExternalOutput")
        stats = nc.dram_tensor("stats", [1, 2], F32, kind="ExternalOutput")

    iota_np = np.tile(np.arange(128, dtype=np.float32), (128, NP_)).astype(BF)
    iota_dr = nc.inline_tensor(iota_np, name="iota_c")
    E = np.eye(128, dtype=np.float32)
    e_dr = nc.inline_tensor(E.astype(BF), name="eI")
    z_dr = nc.inline_tensor(np.zeros((128, 128), np.float32).astype(BF),
                            name="zI")
    if dual:
        w1e_dr = nc.inline_tensor((w1 * E).astype(BF), name="w1e")
    if mode in ('dual', 's2'):
        w2e_dr = nc.inline_tensor((w2 * E).astype(BF), name="w2e")
        ones_dr = nc.inline_tensor(np.ones((128, 1), np.float32), name="ones")

    with TileContext(nc) as tc:
        with (
            tc.tile_pool(name="big", bufs=1) as bigp,
            tc.tile_pool(name="aux", bufs=2) as auxp,
            tc.tile_pool(name="gat", bufs=6) as gatp,
            tc.tile_pool(name="gb", bufs=4) as gbp,
            tc.tile_pool(name="sS", bufs=2) as sSp,
            tc.tile_pool(name="fin", bufs=2) as finp,
            tc.tile_pool(name="rng", bufs=2) as rngp,
            tc.tile_pool(name="p12", bufs=1, space="PSUM") as p12p,
            tc.tile_pool(name="pg", bufs=1, space="PSUM") as pgp,
            tc.tile_pool(name="pst", bufs=1, space="PSUM") as pstp,
        ):
            iota_t = bigp.tile([128, NP_ * 128], BF16, tag='iota',
                               name='iota')
            nc.sync.dma_start(out=iota_t[:], in_=iota_dr[:])
            e_t = bigp.tile([128, 128], BF16, tag='eI', name='eI')
            nc.sync.dma_start(out=e_t[:], in_=e_dr[:])
            z_t = bigp.tile([128, 128], BF16, tag='zI', name='zI')
            nc.sync.dma_start(out=z_t[:], in_=z_dr[:])
            if dual:
                w1e_t = bigp.tile([128, 128], BF16, tag='w1e', name='w1e')
                nc.sync.dma_start(out=w1e_t[:], in_=w1e_dr[:])
            if mode in ('dual', 's2'):
                w2e_t = bigp.tile([128, 128], BF16, tag='w2e', name='w2e')
                nc.sync.dma_start(out=w2e_t[:], in_=w2e_dr[:])
                ones_t = bigp.tile([128, 1], F32, tag='ones', name='ones')
                nc.sync.dma_start(out=ones_t[:], in_=ones_dr[:])
                ssq_acc = bigp.tile([128, 512], F32, tag='ssq', name='ssq')
                nc.vector.memset(ssq_acc[:], 0.0)

            idx_sb = pos_sb = val_sb = None
            t_global = 0
            for rr in range(plan['nr']):
                t_lo, t_hi = plan['range_tiles'][rr]
                # psum tiles for this range
                p12 = [p12p.tile([128, 512], F32, tag=f"p12_{k}",
                                 name=f"p12_{k}") for k in range(n12)]
                if mode in ('dual', 's2'):
                    npg = (rcols * 64 + 511) // 512
                    pg = [pgp.tile([128, 512], F32, tag=f"pg_{k}",
                                   name=f"pg_{k}") for k in range(npg)]
                # whole-bank start openers (start zeroing is 2KB-granular)
                for pt_ in p12:
                    nc.tensor.matmul(out=pt_[:], lhsT=z_t[:],
                                     rhs=iota_t[:, :512], start=True,
                                     stop=False, skip_group_check=True)
                if mode in ('dual', 's2'):
                    for pgt_ in pg:
                        nc.tensor.matmul(out=pgt_[:], lhsT=z_t[:],
                                         rhs=iota_t[:, :512], start=True,
                                         stop=False, skip_group_check=True)
                for t in range(t_lo, t_hi):
                    j = t % 8
                    if j == 0:
                        nchunk = min(8, T - t)
                        idx_sb = auxp.tile([128, 8, TSLOT // 16], I16,
                                           tag="idx", name="idx")
                        pos_sb = auxp.tile([128, 8, NP_], BF16, tag="pos",
                                           name="pos")
                        val_sb = auxp.tile([128, 8, TSLOT // 128], F32,
                                           tag="val", name="val")
                        nc.sync.dma_start(
                            out=idx_sb[:, :nchunk, :],
                            in_=idx_d[t:t + nchunk].rearrange(
                                "t p q -> p t q"))
                        nc.sync.dma_start(
                            out=pos_sb[:, :nchunk, :],
                            in_=pos_d[t:t + nchunk].rearrange(
                                "t p q -> p t q"))
                        nc.sync.dma_start(
                            out=val_sb[:, :nchunk, :],
                            in_=val_d[t:t + nchunk].rearrange(
                                "t p q -> p t q"))
                    g_t = gatp.tile([128, TSLOT // 128, WSRC],
                                    F32 if mode == 's1' else BF16,
                                    tag="g", name="g")
                    for si, (o, ln, wi) in enumerate(plan['segs'][t]):
                        nc.gpsimd.dma_gather(
                            g_t[:, o // 128:(o + ln) // 128, :],
                            src[plan['win_lo'][wi]:plan['win_hi'][wi], :],
                            idx_sb[:, j, o // 16:(o + ln) // 16],
                            ln, ln, WSRC,
                            single_packet=True,
                            queue_num=(2 * t + si) % 4)
                    g_b = gbp.tile([128, TSLOT // 128, WR], BF16, tag="gb",
                                   name="gb")
                    for gg in range(TSLOT // 128):
                        nc.scalar.mul(out=g_b[:, gg, :],
                                      in_=g_t[:, gg, :WR],
                                      mul=val_sb[:, j, gg:gg + 1])
                    npairs = len(plan['pairs'][t])
                    S_t = sSp.tile([128, NP_ * 128], BF16, tag="S", name="S")
                    if npairs:
                        nc.vector.tensor_tensor(
                            out=S_t[:, :npairs * 128].rearrange(
                                "p (a b) -> p a b", b=128),
                            in0=pos_sb[:, j, :npairs].unsqueeze(
                                2).to_broadcast([128, npairs, 128]),
                            in1=iota_t[:, :npairs * 128].rearrange(
                                "p (a b) -> p a b", b=128),
                            op=AL.is_equal)
                    for (np_i, gg, crr, st, a, b_, g) in plan['pairs'][t]:
                        pt = p12[crr // cpb]
                        cc = crr % cpb
                        nc.tensor.matmul(
                            out=pt[:, cc * WR:(cc + 1) * WR],
                            lhsT=S_t[:, np_i * 128:(np_i + 1) * 128],
                            rhs=g_b[:, gg, :],
                            start=False, stop=False,
                            skip_group_check=True)
                # ---- finale for range rr ----
                rd_sb = finp.tile([128, rcols, D], BF16, tag="rd", name="rd")
                nc.sync.dma_start(out=rd_sb[:],
                                  in_=rd_d[:, rr * rcols:(rr + 1) * rcols, :])
                if dual:
                    w0_sb = finp.tile([128, rcols, D], BF16, tag="w0",
                                      name="w0")
                    nc.sync.dma_start(
                        out=w0_sb[:],
                        in_=w0ue_d[:, rr * rcols:(rr + 1) * rcols, :])
                    d_sb = finp.tile([128, rcols], F32, tag="dv", name="dv")
                    nc.sync.dma_start(
                        out=d_sb[:],
                        in_=dvec_d[:, rr * rcols:(rr + 1) * rcols])
                if mode == 's2':
                    wc_sb = finp.tile([128, rcols, D], BF16, tag="wc",
                                      name="wc")
                    nc.sync.dma_start(
                        out=wc_sb[:],
                        in_=wcomb_d[:, rr * rcols:(rr + 1) * rcols, :])
                if mode in ('s1', 'dual'):
                    g1rng = rngp.tile([128, rcols, D], BF16, tag="g1r",
                                      name="g1r")
                if mode in ('dual', 's2'):
                    gcnrng = rngp.tile([128, rcols, D], BF16, tag="gcr",
                                       name="gcr")
                for b0 in range(0, rcols, cpb):
                    pt = p12[b0 // cpb]
                    ptv = pt[:].rearrange("p (c x) -> p c x", x=WR * 1)
                    # inject resid into L1 sub-cols
                    nc.tensor.matmul(
                        out=ptv[:, :, 0:64],
                        lhsT=e_t[:],
                        rhs=rd_sb[:, b0:b0 + cpb, :],
                        start=False, stop=not dual,
                        skip_group_check=True)
                    if mode == 's1':
                        nc.scalar.activation(
                            out=g1rng[:, b0:b0 + cpb, :],
                            in_=ptv[:, :, 0:64], func=AF.Relu)
                        continue
                    if dual:
                        # g1 = relu(p1)
                        nc.scalar.activation(
                            out=g1rng[:, b0:b0 + cpb, :],
                            in_=ptv[:, :, 0:64], func=AF.Relu)
                        # t2 = g1 * d
                        t2 = finp.tile([128, cpb, D], BF16, tag="t2",
                                       name="t2")
                        nc.vector.tensor_tensor(
                            out=t2[:],
                            in0=g1rng[:, b0:b0 + cpb, :],
                            in1=d_sb[:, b0:b0 + cpb].unsqueeze(
                                2).to_broadcast([128, cpb, D]),
                            op=AL.mult)
                        nc.tensor.matmul(
                            out=ptv[:, :, 64:128],
                            lhsT=e_t[:], rhs=t2[:],
                            start=False, stop=True,
                            skip_group_check=True)
                        g2 = finp.tile([128, cpb, D], BF16, tag="g2",
                                       name="g2")
                        nc.scalar.activation(out=g2[:], in_=ptv[:, :, 64:128],
                                             func=AF.Relu)
                        first_rhs = w0_sb[:, b0:b0 + cpb, :]
                    else:  # s2
                        g2 = finp.tile([128, cpb, D], BF16, tag="g2",
                                       name="g2")
                        nc.scalar.activation(out=g2[:], in_=ptv[:, :, 0:64],
                                             func=AF.Relu)
                        first_rhs = wc_sb[:, b0:b0 + cpb, :]
                    # gcn psum: w0ue/wcomb + (w1 g1) + w2 g2
                    pgt = pg[(b0 * 64) // 512]
                    pgv = pgt[:].rearrange("p (c x) -> p c x", x=64)
                    go = (b0 * 64 % 512) // 64
                    pg_last = (go + cpb) * 64 >= 512 or b0 + cpb >= rcols
                    nc.tensor.matmul(
                        out=pgv[:, go:go + cpb, :],
                        lhsT=e_t[:], rhs=first_rhs,
                        start=False, stop=False,
                        skip_group_check=True)
                    if dual:
                        nc.tensor.matmul(
                            out=pgv[:, go:go + cpb, :],
                            lhsT=w1e_t[:],
                            rhs=g1rng[:, b0:b0 + cpb, :],
                            start=False, stop=False,
                            skip_group_check=True)
                    nc.tensor.matmul(
                        out=pgv[:, go:go + cpb, :],
                        lhsT=w2e_t[:], rhs=g2[:],
                        start=False, stop=pg_last,
                        skip_group_check=True)
                    nc.scalar.activation(out=gcnrng[:, b0:b0 + cpb, :],
                                         in_=pgv[:, go:go + cpb, :],
                                         func=AF.Copy)
                    # ssq accumulate
                    sq = finp.tile([128, cpb, D], BF16, tag="sq", name="sq")
                    nc.vector.tensor_tensor(out=sq[:],
                                            in0=gcnrng[:, b0:b0 + cpb, :],
                                            in1=gcnrng[:, b0:b0 + cpb, :],
                                            op=AL.mult)
                    nc.vector.tensor_tensor(
                        out=ssq_acc[:, :cpb * D],
                        in0=ssq_acc[:, :cpb * D],
                        in1=sq[:].rearrange("p c x -> p (c x)"),
                        op=AL.add)
                if mode in ('s1', 'dual'):
                    nc.sync.dma_start(
                        out=g1_out[:, rr * rcols:(rr + 1) * rcols, :],
                        in_=g1rng[:])
                if mode in ('dual', 's2'):
                    nc.sync.dma_start(
                        out=gcn_out[:, rr * rcols:(rr + 1) * rcols, :],
                        in_=gcnrng[:])

            if mode in ('dual', 's2'):
                red = finp.tile([128, 1], F32, tag="red", name="red")
                nc.vector.tensor_reduce(out=red[:], in_=ssq_acc[:],
                                        axis=mybir.AxisListType.X, op=AL.add)
                sq_ps = pstp.tile([1, 2], F32, space="PSUM", tag='sqps',
                                  name='sqps')
                nc.tensor.matmul(out=sq_ps[:1, 0:1], lhsT=ones_t[:],
                                 rhs=red[:], start=True, stop=True,
                                 skip_group_check=True)
                st_sb = finp.tile([1, 2], F32, tag="st", name="st")
                nc.vector.memset(st_sb[:], 0.0)
                nc.vector.tensor_copy(out=st_sb[:1, 0:1], in_=sq_ps[:1, 0:1])
                nc.sync.dma_start(out=stats[:], in_=st_sb[:])
    nc.finalize()
    return nc


def _build_head(nb, repeat=1):
    """Batch head: leaky-MLP on user/item gcn rows, dot, + biases, sse."""
    nc = bacc.Bacc()
    xu = nc.dram_tensor("xu", [D, nb], F32, kind="ExternalInput")
    xi = nc.dram_tensor("xi", [D, nb], F32, kind="ExternalInput")
    fw1t = nc.dram_tensor("fw1t", [D, 2 * D], F32, kind="ExternalInput")
    fb1 = nc.dram_tensor("fb1", [2 * D, 1], F32, kind="ExternalInput")
    fw2t = nc.dram_tensor("fw2t", [2 * D, D], F32, kind="ExternalInput")
    fb2 = nc.dram_tensor("fb2", [D, 1], F32, kind="ExternalInput")
    bsum = nc.dram_tensor("bsum", [1, nb], F32, kind="ExternalInput")
    rat = nc.dram_tensor("rat", [1, nb], F32, kind="ExternalInput")
    out = nc.dram_tensor("out", [1, 1], F32, kind="ExternalOutput")
    ones_dr = nc.inline_tensor(np.ones((D, 1), np.float32), name="ones_h")

    with TileContext(nc) as tc:
        with (
            tc.tile_pool(name="sb", bufs=1) as sp,
            tc.tile_pool(name="wk", bufs=2) as wk,
            tc.tile_pool(name="ps", bufs=2, space="PSUM") as psp,
        ):
            xu_t = sp.tile([D, nb], F32, tag='xu', name='xu')
            xi_t = sp.tile([D, nb], F32, tag='xi', name='xi')
            w1 = sp.tile([D, 2 * D], F32, tag='w1', name='w1')
            b1 = sp.tile([2 * D, 1], F32, tag='b1', name='b1')
            w2 = sp.tile([2 * D, D], F32, tag='w2', name='w2')
            b2 = sp.tile([D, 1], F32, tag='b2', name='b2')
            on = sp.tile([D, 1], F32, tag='on', name='on')
            bs = sp.tile([1, nb], F32, tag='bs', name='bs')
            rt = sp.tile([1, nb], F32, tag='rt', name='rt')
            for t_, d_ in [(xu_t, xu), (xi_t, xi), (w1, fw1t), (b1, fb1),
                           (w2, fw2t), (b2, fb2), (on, ones_dr), (bs, bsum),
                           (rt, rat)]:
                nc.sync.dma_start(out=t_[:], in_=d_[:])

            for _rep in range(repeat):
                outs = []
                for (xt, side) in [(xu_t, 0), (xi_t, 1)]:
                    h_all = sp.tile([2 * D, nb], F32, tag=f"h{side}")
                    for n0 in range(0, nb, 512):
                        nn = min(512, nb - n0)
                        hp = psp.tile([128, 512], F32, tag="hp", space="PSUM")
                        nc.tensor.matmul(out=hp[:, :nn], lhsT=w1[:],
                                         rhs=xt[:, n0:n0 + nn],
                                         start=True, stop=True)
                        sl = h_all[:, n0:n0 + nn]
                        nc.vector.tensor_scalar(out=sl, in0=hp[:, :nn],
                                                scalar1=b1[:, 0:1],
                                                scalar2=None, op0=AL.add)
                        t_ = wk.tile([2 * D, 512], F32, tag="lk", name="lk")
                        nc.vector.tensor_scalar(out=t_[:, :nn], in0=sl,
                                                scalar1=0.1, scalar2=None,
                                                op0=AL.mult)
                        nc.vector.tensor_tensor(out=sl, in0=sl,
                                                in1=t_[:, :nn], op=AL.max)
                    o_all = sp.tile([D, nb], F32, tag=f"o{side}")
                    for n0 in range(0, nb, 512):
                        nn = min(512, nb - n0)
                        op_ = psp.tile([D, 512], F32, tag="op", space="PSUM")
                        nc.tensor.matmul(out=op_[:, :nn], lhsT=w2[:],
                                         rhs=h_all[:, n0:n0 + nn],
                                         start=True, stop=True)
                        sl = o_all[:, n0:n0 + nn]
                        nc.vector.tensor_scalar(out=sl, in0=op_[:, :nn],
                                                scalar1=b2[:, 0:1],
                                                scalar2=None, op0=AL.add)
                        t_ = wk.tile([D, 512], F32, tag="lk2", name="lk2")
                        nc.vector.tensor_scalar(out=t_[:, :nn], in0=sl,
                                                scalar1=0.1, scalar2=None,
                                                op0=AL.mult)
                        nc.vector.tensor_tensor(out=sl, in0=sl,
                                                in1=t_[:, :nn], op=AL.max)
                    outs.append(o_all)

                prod = sp.tile([D, nb], F32, tag='prod', name='prod')
                nc.vector.tensor_tensor(out=prod[:], in0=outs[0][:],
                                        in1=outs[1][:], op=AL.mult)
                pred = sp.tile([1, nb], F32, tag='pred', name='pred')
                for n0 in range(0, nb, 512):
                    nn = min(512, nb - n0)
                    pp = psp.tile([1, 512], F32, tag="pp", space="PSUM")
                    nc.tensor.matmul(out=pp[:1, :nn], lhsT=on[:],
                                     rhs=prod[:, n0:n0 + nn],
                                     start=True, stop=True)
                    nc.vector.tensor_copy(out=pred[:, n0:n0 + nn],
                                          in_=pp[:1, :nn])
                nc.vector.tensor_tensor(out=pred[:], in0=pred[:], in1=bs[:],
                                        op=AL.add)
                nc.vector.tensor_tensor(out=pred[:], in0=pred[:], in1=rt[:],
                                        op=AL.subtract)
                nc.vector.tensor_tensor(out=pred[:], in0=pred[:], in1=pred[:],
                                        op=AL.mult)
                sse = sp.tile([1, 1], F32, tag='sse', name='sse')
                nc.vector.tensor_reduce(out=sse[:], in_=pred[:],
                                        axis=mybir.AxisListType.X, op=AL.add)
                nc.sync.dma_start(out=out[:], in_=sse[:])
    nc.finalize()
    return nc


# ------------------------------------------------------------ orchestration --

def _to_storage(arr, mp, width=None):
    w = arr.shape[1] if arr.ndim > 1 else 1
    out = np.zeros((mp['n_pad'], width or w), arr.dtype)
    out[mp['storage'], :w] = arr.reshape(len(arr), w)
    return out


def _shard(full, mp, core, C_pad, dtype):
    """storage-flat [n_pad, w] -> per-core [128, C_pad, w] (pad cols zero)."""
    C = mp['C']
    blk = full[core * mp['rows_per_core']:(core + 1) * mp['rows_per_core']]
    w = blk.shape[1]
    out = np.zeros((128, C_pad, w), dtype)
    out[:, :C, :] = blk.reshape(128, C, w)
    return np.ascontiguousarray(out)


def _unshard(results, key, mp, C_pad):
    """per-core [128, C_pad, w] -> storage-flat [n_pad, w] float32."""
    C = mp['C']
    outs = []
    for r in results:
        a = np.asarray(r[key])[:, :C, :].astype(np.float32)
        outs.append(a.reshape(mp['rows_per_core'], -1))
    return np.concatenate(outs, axis=0)


def _run(nc, in_maps, label):
    import time
    t0 = time.time()
    res = run_bass_kernel_spmd(nc, in_maps, core_ids=list(range(len(in_maps))))
    wall = time.time() - t0
    _EXEC_NS.setdefault("walls", []).append((label, wall))
    _EXEC_NS.setdefault("launches", []).append((label, nc, in_maps))
    return res.results


def kernel(ui_rows, ui_cols, ui_vals, iu_vals, d_i, d_j,
           embed_user, embed_item, add_w, fw1, fb1, fw2, fb2,
           user_bias, item_bias, avg_rating, user0, item_i0, ratings):
    ui_rows = np.asarray(ui_rows)
    ui_cols = np.asarray(ui_cols)
    mu = _side_mapping(U)
    mi = _side_mapping(I)
    w = np.asarray(add_w, np.float32)[0]

    planA = _plan_v2(mu['core'][ui_rows], mu['local'][ui_rows],
                     mi['storage'][ui_cols], np.asarray(ui_vals, np.float32),
                     mu['C'], mi['n_pad'], rcols=16)
    planB1 = _plan_v2(mi['core'][ui_cols], mi['local'][ui_cols],
                      mu['storage'][ui_rows], np.asarray(iu_vals, np.float32),
                      mi['C'], mu['n_pad'], rcols=48)
    planB2 = _plan_v2(mi['core'][ui_cols], mi['local'][ui_cols],
                      mu['storage'][ui_rows], np.asarray(iu_vals, np.float32),
                      mi['C'], mu['n_pad'], rcols=24)

    ncB1 = _build_side_v2(planB1, mu['n_pad'], 's1')
    ncA12 = _build_side_v2(planA, mi['n_pad'], 'dual', w1=float(w[1]),
                           w2=float(w[2]))
    ncB2 = _build_side_v2(planB2, mu['n_pad'], 's2', w2=float(w[2]))

    ue = np.asarray(embed_user, np.float32)
    ie = np.asarray(embed_item, np.float32)
    d_i = np.asarray(d_i, np.float32)
    d_j = np.asarray(d_j, np.float32)

    eu_st = _to_storage(ue, mu)                       # [n_pad,64] f32
    # B1: src=ue f32; rd = ie*d_j bf16 (item side)
    rdB = (ie * d_j[:, None]).astype(BF)
    rdB_st = _to_storage(rdB, mi)
    mapsB1 = []
    for c in range(NCORES):
        mapsB1.append({
            "src": eu_st,
            "idx": planB1['idx'][c], "pos": planB1['pos'][c],
            "val": planB1['val'][c],
            "rd": _shard(rdB_st, mi, c, planB1['C_pad'], BF),
        })
    rB1 = _run(ncB1, mapsB1, "B1")
    g1i = _unshard(rB1, "g1_out", mi, planB1['C_pad'])  # storage-flat f32

    # A12: src=[ie|g1_i] bf16; rd=ue*d_i; w0ue; dvec=d_i
    srcA = np.zeros((mi['n_pad'], 128), BF)
    srcA[:, :64] = _to_storage(ie.astype(BF), mi)
    srcA[:, 64:] = g1i.astype(BF)
    rdA_st = _to_storage((ue * d_i[:, None]).astype(BF), mu)
    w0ue_st = _to_storage((w[0] * ue).astype(BF), mu)
    di_st = _to_storage(d_i.astype(np.float32), mu)
    mapsA = []
    for c in range(NCORES):
        mapsA.append({
            "src": srcA,
            "idx": planA['idx'][c], "pos": planA['pos'][c],
            "val": planA['val'][c],
            "rd": _shard(rdA_st, mu, c, planA['C_pad'], BF),
            "w0ue": _shard(w0ue_st, mu, c, planA['C_pad'], BF),
            "dvec": np.ascontiguousarray(_shard(di_st, mu, c,
                           planA['C_pad'], np.float32)[:, :, 0]),
        })
    rA = _run(ncA12, mapsA, "A12")
    g1u = _unshard(rA, "g1_out", mu, planA['C_pad'])
    gcnu = _unshard(rA, "gcn_out", mu, planA['C_pad'])
    ssq_u = sum(float(r["stats"][0, 0]) for r in rA)

    # B2: src=[g1_u|0] bf16; rd = g1_i*d_j; wcomb = w0*ie + w1*g1_i
    srcB2 = np.zeros((mu['n_pad'], 128), BF)
    srcB2[:, :64] = g1u.astype(BF)
    rd2_st = np.zeros((mi['n_pad'], 64), BF)
    rd2_st[:] = (g1i * _to_storage(d_j.reshape(-1, 1), mi)).astype(BF)
    wcomb_st = (w[0] * _to_storage(ie, mi) + w[1] * g1i).astype(BF)
    mapsB2 = []
    for c in range(NCORES):
        mapsB2.append({
            "src": srcB2,
            "idx": planB2['idx'][c], "pos": planB2['pos'][c],
            "val": planB2['val'][c],
            "rd": _shard(rd2_st, mi, c, planB2['C_pad'], BF),
            "wcomb": _shard(wcomb_st, mi, c, planB2['C_pad'], BF),
        })
    rB2 = _run(ncB2, mapsB2, "B2")
    gcni = _unshard(rB2, "gcn_out", mi, planB2['C_pad'])
    ssq_i = sum(float(r["stats"][0, 0]) for r in rB2)

    # head
    nb = B // NCORES
    user0 = np.asarray(user0)
    item_i0 = np.asarray(item_i0)
    xu_rows = gcnu[mu['storage'][user0]]
    xi_rows = gcni[mi['storage'][item_i0]]
    bsum = (np.asarray(user_bias, np.float32)[user0, 0]
            + np.asarray(item_bias, np.float32)[item_i0, 0]
            + np.float32(np.asarray(avg_rating, np.float32)[0]))
    nch = _build_head(nb)
    _EXEC_NS['headnb'] = nb
    hmaps = []
    for c in range(NCORES):
        sl = slice(c * nb, (c + 1) * nb)
        hmaps.append({
            "xu": np.ascontiguousarray(xu_rows[sl].T),
            "xi": np.ascontiguousarray(xi_rows[sl].T),
            "fw1t": np.ascontiguousarray(np.asarray(fw1, np.float32).T),
            "fb1": np.asarray(fb1, np.float32).reshape(2 * D, 1),
            "fw2t": np.ascontiguousarray(np.asarray(fw2, np.float32).T),
            "fb2": np.asarray(fb2, np.float32).reshape(D, 1),
            "bsum": bsum[sl].reshape(1, nb),
            "rat": np.asarray(ratings, np.float32)[sl].reshape(1, nb),
        })
    rH = _run(nch, hmaps, "H")
    sse = sum(float(r["out"][0, 0]) for r in rH)

    loss = (sse / B + LAM * ssq_u / (U * D) + LAM * ssq_i / (I * D))
    return np.float32(loss)


# revision 10
# speedup vs baseline: 1.1331x; 1.1331x over previous
"""GCN-3 Trainium2 kernel v2 — 3 SpMM launches + head, 8 cores.

Structure vs baseline:
- B1 (dest=items, gathers ue f32) -> A12 (dest=users, dual-layer gather of
  [ie|g1_i] bf16, computes g1_u AND g2_u + gcn_u in one pass) -> B2
  (dest=items, gathers g1_u bf16-padded, computes g2_i + gcn_i) -> head.
- 64-dest half-cells: S matrices [slot,64]; two halves packed vertically in
  PSUM via column-tiled matmuls (tile_position (0,0)/(0,64)).
- PSUM-resident accumulation over col-ranges (no SBUF acc, no per-window
  evictions); residual/gcn combine injected via static identity matmuls,
  relu evictions on ACT straight from PSUM.
- val-scale on ACT (fused with f32->bf16 cast where applicable).
"""
import os
import sys

sys.path.insert(0, '/opt/trn_rl_repo')
os.environ.setdefault("MYCRO_LOCAL_CACHE", "1")
os.environ.setdefault("NEURON_RT_RESET_CORES", "1")

import numpy as np
import ml_dtypes
import concourse.bacc as bacc
import concourse.bass as bass
import concourse.mybir as mybir
from concourse.tile import TileContext
from concourse.bass_utils import run_bass_kernel_spmd

F32 = mybir.dt.float32
BF16 = mybir.dt.bfloat16
I16 = mybir.dt.int16
AL = mybir.AluOpType
AF = mybir.ActivationFunctionType
BF = ml_dtypes.bfloat16

U = 359347
I = 292589
D = 64
B = 32768
LAM = 0.001

WIN = 32768
TSLOT = 2048
NCORES = 8

_EXEC_NS = {"total": 0}


# ----------------------------------------------------------------- planner --

def _side_mapping(n_orig):
    nd = int(np.ceil(n_orig / NCORES))
    C = int(np.ceil(nd / 128))
    rows_per_core = 128 * C
    n_pad = NCORES * rows_per_core
    ids = np.arange(n_orig)
    core = np.minimum(ids // nd, NCORES - 1)
    local = ids - core * nd
    storage = core * rows_per_core + (local % 128) * C + (local // 128)
    return dict(nd=nd, C=C, rows_per_core=rows_per_core, n_pad=n_pad,
                core=core.astype(np.int32), local=local.astype(np.int64),
                storage=storage.astype(np.int64))


def _plan_v2(dest_core, dest_local, src_store, vals, C, ns_pad, rcols):
    """Range/half-cell plan. Cells keyed (range, window, col-in-range, half).
    Window blocks padded to 128 slots; ranges padded to TSLOT."""
    nw = int(np.ceil(ns_pad / WIN))
    nr = int(np.ceil(C / rcols))
    C_pad = nr * rcols

    col = (dest_local // 128).astype(np.int64)
    hposv = (dest_local % 128).astype(np.float32)
    w = (src_store // WIN).astype(np.int64)
    idx16 = (src_store % WIN).astype(np.int16)

    r = col // rcols
    cr = col % rcols
    ncell = nr * nw * rcols
    cid = (r * nw + w) * rcols + cr
    keys = dest_core.astype(np.int64) * ncell + cid
    cnt = np.bincount(keys, minlength=NCORES * ncell).reshape(NCORES, ncell)
    npad = cnt.max(axis=0).astype(np.int64)
    # force >=1 slot in the w==0 cell of every (r, cr) so every psum slice
    # has at least one S-matmul (injections can then use start=False).
    w0cells = (np.arange(nr)[:, None] * nw * rcols
               + np.arange(rcols)[None, :]).ravel()
    npad[w0cells] = np.maximum(npad[w0cells], 1)

    # offsets: iterate ranges, windows; pad window block to 128, range to TSLOT
    off = np.zeros(ncell, np.int64)
    win_of = []            # per 128-slot block, the window id
    base = 0
    range_tile_lo = []
    for rr in range(nr):
        r_start = base
        for ww in range(nw):
            c0 = (rr * nw + ww) * rcols
            cells = np.arange(c0, c0 + rcols)
            sizes = npad[cells]
            co = np.concatenate([[0], np.cumsum(sizes)])
            off[cells] = base + co[:-1]
            blk = int(co[-1])
            blk_pad = (blk + 127) // 128 * 128
            win_of.extend([ww] * (blk_pad // 128))
            base += blk_pad
        # pad range to TSLOT
        rng = base - r_start
        rng_pad = (rng + TSLOT - 1) // TSLOT * TSLOT
        lastw = win_of[-1] if win_of else 0
        win_of.extend([lastw] * ((rng_pad - rng) // 128))
        base += rng_pad - rng
        range_tile_lo.append((r_start // TSLOT, base // TSLOT))
    total = int(base)
    T = total // TSLOT
    win_of = np.asarray(win_of, np.int64)  # [total//128]

    # slot assignment
    order = np.lexsort((dest_local, keys))
    sk = keys[order]
    first = np.r_[True, sk[1:] != sk[:-1]]
    grp_start = np.flatnonzero(first)
    grp_id = np.cumsum(first) - 1
    rank = np.arange(len(sk)) - grp_start[grp_id]
    e_core = dest_core[order]
    slot = off[cid[order]] + rank
    slot_idx = np.zeros((NCORES, total), np.int16)
    slot_val = np.zeros((NCORES, total), np.float32)
    slot_pos = np.full((NCORES, total), -10000.0, np.float32)
    slot_idx[e_core, slot] = idx16[order]
    slot_val[e_core, slot] = vals[order]
    slot_pos[e_core, slot] = hposv[order]

    # pairs per tile: pieces of cells within 128-slot groups
    cell_lo = off
    cell_hi = off + npad
    # order cells by lo for sweep
    live_cells = np.flatnonzero(npad > 0)
    lo_sorted = live_cells[np.argsort(cell_lo[live_cells])]
    pairs_all = [[] for _ in range(T)]
    seen_first = np.zeros(ncell, bool)
    # decode cid -> (r, w, cr)
    def _dec(c):
        crr = c % rcols
        c //= rcols
        ww = c % nw
        rr = c // nw
        return rr, ww, crr

    started = set()  # (r, cr) psum slices already started
    np_counts = np.zeros(T, np.int64)
    for c_ in lo_sorted:
        lo = int(cell_lo[c_]); hi = int(cell_hi[c_])
        rr, ww, crr = _dec(int(c_))
        key = (rr, crr)
        for g in range(lo // 128, (hi - 1) // 128 + 1):
            glo = max(lo, g * 128)
            ghi = min(hi, (g + 1) * 128)
            t = g // (TSLOT // 128)
            gg = g % (TSLOT // 128)
            st = key not in started
            started.add(key)
            pairs_all[t].append((int(np_counts[t]), gg, crr, st,
                                 glo - g * 128, ghi - g * 128, g))
            np_counts[t] += 1
    NPMAX = int((np_counts.max() + 7) // 8 * 8)

    posq = np.full((NCORES, T, 128, NPMAX), -10000.0, np.float32)
    for t in range(T):
        for (np_i, gg, crr, st, a, b, g) in pairs_all[t]:
            ks = np.arange(a, b)
            sl = np.arange(g * 128 + a, g * 128 + b)
            posq[:, t, ks, np_i] = slot_pos[:, sl]

    # gather segments per tile: runs of equal window, <=1024, 128-aligned
    segs_all = []
    for t in range(T):
        blocks = win_of[t * 16:(t + 1) * 16]
        segs = []
        s0 = 0
        for b_ in range(1, 17):
            if b_ == 16 or blocks[b_] != blocks[s0]:
                ln = (b_ - s0) * 128
                o = s0 * 128
                while ln > 0:
                    take = min(ln, 1024)
                    segs.append((o, take, int(blocks[s0])))
                    o += take
                    ln -= take
                s0 = b_
        segs_all.append(segs)

    idx_t = slot_idx.reshape(NCORES, T, TSLOT // 16, 16).transpose(0, 1, 3, 2)
    idx_t = np.ascontiguousarray(np.tile(idx_t, (1, 1, 8, 1)))
    val_t = np.ascontiguousarray(
        slot_val.reshape(NCORES, T, TSLOT // 128, 128).transpose(0, 1, 3, 2))
    return dict(T=T, nw=nw, nr=nr, C_pad=C_pad, rcols=rcols, NPMAX=NPMAX,
                pairs=pairs_all, segs=segs_all, idx=idx_t, val=val_t,
                pos=np.ascontiguousarray(posq).astype(BF),
                range_tiles=range_tile_lo,
                win_lo=[wi * WIN for wi in range(nw)],
                win_hi=[min((wi + 1) * WIN, ns_pad) for wi in range(nw)])


# ----------------------------------------------------------------- builder --

def _build_side_v2(plan, ns_pad, mode, w1=0.0, w2=0.0):
    """mode: 's1' (f32 src, g1 out), 'dual' (bf16x128 src, g1+gcn+ssq out),
    's2' (bf16x128 padded src, gcn+ssq out)."""
    T, NP_, rcols = plan['T'], plan['NPMAX'], plan['rcols']
    C_pad = plan['C_pad']
    dual = mode == 'dual'
    WSRC = 128 if mode != 's1' else 64     # src row elems
    WR = 128 if dual else 64               # rhs width per matmul
    cpb = 4 if dual else 8                 # cols per psum bank
    n12 = rcols // cpb                     # spmm psum banks per range

    nc = bacc.Bacc(num_swdge_queues=4)
    src = nc.dram_tensor("src", [ns_pad, WSRC], F32 if mode == 's1' else BF16,
                         kind="ExternalInput")
    idx_d = nc.dram_tensor("idx", [T, 128, TSLOT // 16], I16,
                           kind="ExternalInput")
    pos_d = nc.dram_tensor("pos", [T, 128, NP_], BF16, kind="ExternalInput")
    val_d = nc.dram_tensor("val", [T, 128, TSLOT // 128], F32,
                           kind="ExternalInput")
    rd_d = nc.dram_tensor("rd", [128, C_pad, D], BF16, kind="ExternalInput")
    if dual:
        w0ue_d = nc.dram_tensor("w0ue", [128, C_pad, D], BF16,
                                kind="ExternalInput")
        dvec_d = nc.dram_tensor("dvec", [128, C_pad], F32,
                                kind="ExternalInput")
    if mode == 's2':
        wcomb_d = nc.dram_tensor("wcomb", [128, C_pad, D], BF16,
                                 kind="ExternalInput")
    if mode in ('s1', 'dual'):
        g1_out = nc.dram_tensor("g1_out", [128, C_pad, D], BF16,
                                kind="ExternalOutput")
    if mode in ('dual', 's2'):
        gcn_out = nc.dram_tensor("gcn_out", [128, C_pad, D], BF16,
                                 kind="ExternalOutput")
        stats = nc.dram_tensor("stats", [1, 2], F32, kind="ExternalOutput")

    iota_np = np.tile(np.arange(128, dtype=np.float32), (128, NP_)).astype(BF)
    iota_dr = nc.inline_tensor(iota_np, name="iota_c")
    E = np.eye(128, dtype=np.float32)
    e_dr = nc.inline_tensor(E.astype(BF), name="eI")
    z_dr = nc.inline_tensor(np.zeros((128, 128), np.float32).astype(BF),
                            name="zI")
    if dual:
        w1e_dr = nc.inline_tensor((w1 * E).astype(BF), name="w1e")
    if mode in ('dual', 's2'):
        w2e_dr = nc.inline_tensor((w2 * E).astype(BF), name="w2e")
        ones_dr = nc.inline_tensor(np.ones((128, 1), np.float32), name="ones")

    with TileContext(nc) as tc:
        with (
            tc.tile_pool(name="big", bufs=1) as bigp,
            tc.tile_pool(name="aux", bufs=2) as auxp,
            tc.tile_pool(name="gat", bufs=6) as gatp,
            tc.tile_pool(name="gb", bufs=4) as gbp,
            tc.tile_pool(name="sS", bufs=2) as sSp,
            tc.tile_pool(name="fin", bufs=2) as finp,
            tc.tile_pool(name="rng", bufs=2) as rngp,
            tc.tile_pool(name="p12", bufs=1, space="PSUM") as p12p,
            tc.tile_pool(name="pg", bufs=1, space="PSUM") as pgp,
            tc.tile_pool(name="pst", bufs=1, space="PSUM") as pstp,
        ):
            iota_t = bigp.tile([128, NP_ * 128], BF16, tag='iota',
                               name='iota')
            nc.sync.dma_start(out=iota_t[:], in_=iota_dr[:])
            e_t = bigp.tile([128, 128], BF16, tag='eI', name='eI')
            nc.sync.dma_start(out=e_t[:], in_=e_dr[:])
            z_t = bigp.tile([128, 128], BF16, tag='zI', name='zI')
            nc.sync.dma_start(out=z_t[:], in_=z_dr[:])
            if dual:
                w1e_t = bigp.tile([128, 128], BF16, tag='w1e', name='w1e')
                nc.sync.dma_start(out=w1e_t[:], in_=w1e_dr[:])
            if mode in ('dual', 's2'):
                w2e_t = bigp.tile([128, 128], BF16, tag='w2e', name='w2e')
                nc.sync.dma_start(out=w2e_t[:], in_=w2e_dr[:])
                ones_t = bigp.tile([128, 1], F32, tag='ones', name='ones')
                nc.sync.dma_start(out=ones_t[:], in_=ones_dr[:])
                ssq_acc = bigp.tile([128, 512], F32, tag='ssq', name='ssq')
                nc.vector.memset(ssq_acc[:], 0.0)

            idx_sb = pos_sb = val_sb = None
            t_global = 0
            for rr in range(plan['nr']):
                t_lo, t_hi = plan['range_tiles'][rr]
                # psum tiles for this range
                p12 = [p12p.tile([128, 512], F32, tag=f"p12_{k}",
                                 name=f"p12_{k}") for k in range(n12)]
                if mode in ('dual', 's2'):
                    npg = (rcols * 64 + 511) // 512
                    pg = [pgp.tile([128, 512], F32, tag=f"pg_{k}",
                                   name=f"pg_{k}") for k in range(npg)]
                # whole-bank start openers (start zeroing is 2KB-granular)
                for pt_ in p12:
                    nc.tensor.matmul(out=pt_[:], lhsT=z_t[:],
                                     rhs=iota_t[:, :512], start=True,
                                     stop=False, skip_group_check=True)
                if mode in ('dual', 's2'):
                    for pgt_ in pg:
                        nc.tensor.matmul(out=pgt_[:], lhsT=z_t[:],
                                         rhs=iota_t[:, :512], start=True,
                                         stop=False, skip_group_check=True)
                for t in range(t_lo, t_hi):
                    j = t % 8
                    if j == 0:
                        nchunk = min(8, T - t)
                        idx_sb = auxp.tile([128, 8, TSLOT // 16], I16,
                                           tag="idx", name="idx")
                        pos_sb = auxp.tile([128, 8, NP_], BF16, tag="pos",
                                           name="pos")
                        val_sb = auxp.tile([128, 8, TSLOT // 128], F32,
                                           tag="val", name="val")
                        nc.sync.dma_start(
                            out=idx_sb[:, :nchunk, :],
                            in_=idx_d[t:t + nchunk].rearrange(
                                "t p q -> p t q"))
                        nc.sync.dma_start(
                            out=pos_sb[:, :nchunk, :],
                            in_=pos_d[t:t + nchunk].rearrange(
                                "t p q -> p t q"))
                        nc.sync.dma_start(
                            out=val_sb[:, :nchunk, :],
                            in_=val_d[t:t + nchunk].rearrange(
                                "t p q -> p t q"))
                    g_t = gatp.tile([128, TSLOT // 128, WSRC],
                                    F32 if mode == 's1' else BF16,
                                    tag="g", name="g")
                    for si, (o, ln, wi) in enumerate(plan['segs'][t]):
                        nc.gpsimd.dma_gather(
                            g_t[:, o // 128:(o + ln) // 128, :],
                            src[plan['win_lo'][wi]:plan['win_hi'][wi], :],
                            idx_sb[:, j, o // 16:(o + ln) // 16],
                            ln, ln, WSRC,
                            single_packet=True,
                            queue_num=(2 * t + si) % 4)
                    g_b = gbp.tile([128, TSLOT // 128, WR], BF16, tag="gb",
                                   name="gb")
                    NACT = 11
                    for gg in range(NACT):
                        nc.scalar.mul(out=g_b[:, gg, :],
                                      in_=g_t[:, gg, :WR],
                                      mul=val_sb[:, j, gg:gg + 1])
                    nc.vector.tensor_tensor(
                        out=g_b[:, NACT:, :],
                        in0=g_t[:, NACT:, :WR],
                        in1=val_sb[:, j, NACT:].unsqueeze(2).to_broadcast(
                            [128, 16 - NACT, WR]),
                        op=AL.mult)
                    npairs = len(plan['pairs'][t])
                    S_t = sSp.tile([128, NP_ * 128], BF16, tag="S", name="S")
                    if npairs:
                        nc.vector.tensor_tensor(
                            out=S_t[:, :npairs * 128].rearrange(
                                "p (a b) -> p a b", b=128),
                            in0=pos_sb[:, j, :npairs].unsqueeze(
                                2).to_broadcast([128, npairs, 128]),
                            in1=iota_t[:, :npairs * 128].rearrange(
                                "p (a b) -> p a b", b=128),
                            op=AL.is_equal)
                    for (np_i, gg, crr, st, a, b_, g) in plan['pairs'][t]:
                        pt = p12[crr // cpb]
                        cc = crr % cpb
                        nc.tensor.matmul(
                            out=pt[:, cc * WR:(cc + 1) * WR],
                            lhsT=S_t[:, np_i * 128:(np_i + 1) * 128],
                            rhs=g_b[:, gg, :],
                            start=False, stop=False,
                            skip_group_check=True)
                # ---- finale for range rr ----
                rd_sb = finp.tile([128, rcols, D], BF16, tag="rd", name="rd")
                nc.sync.dma_start(out=rd_sb[:],
                                  in_=rd_d[:, rr * rcols:(rr + 1) * rcols, :])
                if dual:
                    w0_sb = finp.tile([128, rcols, D], BF16, tag="w0",
                                      name="w0")
                    nc.sync.dma_start(
                        out=w0_sb[:],
                        in_=w0ue_d[:, rr * rcols:(rr + 1) * rcols, :])
                    d_sb = finp.tile([128, rcols], F32, tag="dv", name="dv")
                    nc.sync.dma_start(
                        out=d_sb[:],
                        in_=dvec_d[:, rr * rcols:(rr + 1) * rcols])
                if mode == 's2':
                    wc_sb = finp.tile([128, rcols, D], BF16, tag="wc",
                                      name="wc")
                    nc.sync.dma_start(
                        out=wc_sb[:],
                        in_=wcomb_d[:, rr * rcols:(rr + 1) * rcols, :])
                if mode in ('s1', 'dual'):
                    g1rng = rngp.tile([128, rcols, D], BF16, tag="g1r",
                                      name="g1r")
                if mode in ('dual', 's2'):
                    gcnrng = rngp.tile([128, rcols, D], BF16, tag="gcr",
                                       name="gcr")
                for b0 in range(0, rcols, cpb):
                    pt = p12[b0 // cpb]
                    ptv = pt[:].rearrange("p (c x) -> p c x", x=WR * 1)
                    # inject resid into L1 sub-cols
                    nc.tensor.matmul(
                        out=ptv[:, :, 0:64],
                        lhsT=e_t[:],
                        rhs=rd_sb[:, b0:b0 + cpb, :],
                        start=False, stop=not dual,
                        skip_group_check=True)
                    if mode == 's1':
                        nc.scalar.activation(
                            out=g1rng[:, b0:b0 + cpb, :],
                            in_=ptv[:, :, 0:64], func=AF.Relu)
                        continue
                    if dual:
                        # g1 = relu(p1)
                        nc.scalar.activation(
                            out=g1rng[:, b0:b0 + cpb, :],
                            in_=ptv[:, :, 0:64], func=AF.Relu)
                        # t2 = g1 * d
                        t2 = finp.tile([128, cpb, D], BF16, tag="t2",
                                       name="t2")
                        nc.vector.tensor_tensor(
                            out=t2[:],
                            in0=g1rng[:, b0:b0 + cpb, :],
                            in1=d_sb[:, b0:b0 + cpb].unsqueeze(
                                2).to_broadcast([128, cpb, D]),
                            op=AL.mult)
                        nc.tensor.matmul(
                            out=ptv[:, :, 64:128],
                            lhsT=e_t[:], rhs=t2[:],
                            start=False, stop=True,
                            skip_group_check=True)
                        g2 = finp.tile([128, cpb, D], BF16, tag="g2",
                                       name="g2")
                        nc.scalar.activation(out=g2[:], in_=ptv[:, :, 64:128],
                                             func=AF.Relu)
                        first_rhs = w0_sb[:, b0:b0 + cpb, :]
                    else:  # s2
                        g2 = finp.tile([128, cpb, D], BF16, tag="g2",
                                       name="g2")
                        nc.scalar.activation(out=g2[:], in_=ptv[:, :, 0:64],
                                             func=AF.Relu)
                        first_rhs = wc_sb[:, b0:b0 + cpb, :]
                    # gcn psum: w0ue/wcomb + (w1 g1) + w2 g2
                    pgt = pg[(b0 * 64) // 512]
                    pgv = pgt[:].rearrange("p (c x) -> p c x", x=64)
                    go = (b0 * 64 % 512) // 64
                    pg_last = (go + cpb) * 64 >= 512 or b0 + cpb >= rcols
                    nc.tensor.matmul(
                        out=pgv[:, go:go + cpb, :],
                        lhsT=e_t[:], rhs=first_rhs,
                        start=False, stop=False,
                        skip_group_check=True)
                    if dual:
                        nc.tensor.matmul(
                            out=pgv[:, go:go + cpb, :],
                            lhsT=w1e_t[:],
                            rhs=g1rng[:, b0:b0 + cpb, :],
                            start=False, stop=False,
                            skip_group_check=True)
                    nc.tensor.matmul(
                        out=pgv[:, go:go + cpb, :],
                        lhsT=w2e_t[:], rhs=g2[:],
                        start=False, stop=pg_last,
                        skip_group_check=True)
                    nc.scalar.activation(out=gcnrng[:, b0:b0 + cpb, :],
                                         in_=pgv[:, go:go + cpb, :],
                                         func=AF.Copy)
                    # ssq accumulate
                    sq = finp.tile([128, cpb, D], BF16, tag="sq", name="sq")
                    nc.vector.tensor_tensor(out=sq[:],
                                            in0=gcnrng[:, b0:b0 + cpb, :],
                                            in1=gcnrng[:, b0:b0 + cpb, :],
                                            op=AL.mult)
                    nc.vector.tensor_tensor(
                        out=ssq_acc[:, :cpb * D],
                        in0=ssq_acc[:, :cpb * D],
                        in1=sq[:].rearrange("p c x -> p (c x)"),
                        op=AL.add)
                if mode in ('s1', 'dual'):
                    nc.sync.dma_start(
                        out=g1_out[:, rr * rcols:(rr + 1) * rcols, :],
                        in_=g1rng[:])
                if mode in ('dual', 's2'):
                    nc.sync.dma_start(
                        out=gcn_out[:, rr * rcols:(rr + 1) * rcols, :],
                        in_=gcnrng[:])

            if mode in ('dual', 's2'):
                red = finp.tile([128, 1], F32, tag="red", name="red")
                nc.vector.tensor_reduce(out=red[:], in_=ssq_acc[:],
                                        axis=mybir.AxisListType.X, op=AL.add)
                sq_ps = pstp.tile([1, 2], F32, space="PSUM", tag='sqps',
                                  name='sqps')
                nc.tensor.matmul(out=sq_ps[:1, 0:1], lhsT=ones_t[:],
                                 rhs=red[:], start=True, stop=True,
                                 skip_group_check=True)
                st_sb = finp.tile([1, 2], F32, tag="st", name="st")
                nc.vector.memset(st_sb[:], 0.0)
                nc.vector.tensor_copy(out=st_sb[:1, 0:1], in_=sq_ps[:1, 0:1])
                nc.sync.dma_start(out=stats[:], in_=st_sb[:])
    nc.finalize()
    return nc


def _build_head(nb, repeat=1):
    """Batch head: leaky-MLP on user/item gcn rows, dot, + biases, sse."""
    nc = bacc.Bacc()
    xu = nc.dram_tensor("xu", [D, nb], F32, kind="ExternalInput")
    xi = nc.dram_tensor("xi", [D, nb], F32, kind="ExternalInput")
    fw1t = nc.dram_tensor("fw1t", [D, 2 * D], F32, kind="ExternalInput")
    fb1 = nc.dram_tensor("fb1", [2 * D, 1], F32, kind="ExternalInput")
    fw2t = nc.dram_tensor("fw2t", [2 * D, D], F32, kind="ExternalInput")
    fb2 = nc.dram_tensor("fb2", [D, 1], F32, kind="ExternalInput")
    bsum = nc.dram_tensor("bsum", [1, nb], F32, kind="ExternalInput")
    rat = nc.dram_tensor("rat", [1, nb], F32, kind="ExternalInput")
    out = nc.dram_tensor("out", [1, 1], F32, kind="ExternalOutput")
    ones_dr = nc.inline_tensor(np.ones((D, 1), np.float32), name="ones_h")

    with TileContext(nc) as tc:
        with (
            tc.tile_pool(name="sb", bufs=1) as sp,
            tc.tile_pool(name="wk", bufs=2) as wk,
            tc.tile_pool(name="ps", bufs=2, space="PSUM") as psp,
        ):
            xu_t = sp.tile([D, nb], F32, tag='xu', name='xu')
            xi_t = sp.tile([D, nb], F32, tag='xi', name='xi')
            w1 = sp.tile([D, 2 * D], F32, tag='w1', name='w1')
            b1 = sp.tile([2 * D, 1], F32, tag='b1', name='b1')
            w2 = sp.tile([2 * D, D], F32, tag='w2', name='w2')
            b2 = sp.tile([D, 1], F32, tag='b2', name='b2')
            on = sp.tile([D, 1], F32, tag='on', name='on')
            bs = sp.tile([1, nb], F32, tag='bs', name='bs')
            rt = sp.tile([1, nb], F32, tag='rt', name='rt')
            for t_, d_ in [(xu_t, xu), (xi_t, xi), (w1, fw1t), (b1, fb1),
                           (w2, fw2t), (b2, fb2), (on, ones_dr), (bs, bsum),
                           (rt, rat)]:
                nc.sync.dma_start(out=t_[:], in_=d_[:])

            for _rep in range(repeat):
                outs = []
                for (xt, side) in [(xu_t, 0), (xi_t, 1)]:
                    h_all = sp.tile([2 * D, nb], F32, tag=f"h{side}")
                    for n0 in range(0, nb, 512):
                        nn = min(512, nb - n0)
                        hp = psp.tile([128, 512], F32, tag="hp", space="PSUM")
                        nc.tensor.matmul(out=hp[:, :nn], lhsT=w1[:],
                                         rhs=xt[:, n0:n0 + nn],
                                         start=True, stop=True)
                        sl = h_all[:, n0:n0 + nn]
                        nc.vector.tensor_scalar(out=sl, in0=hp[:, :nn],
                                                scalar1=b1[:, 0:1],
                                                scalar2=None, op0=AL.add)
                        t_ = wk.tile([2 * D, 512], F32, tag="lk", name="lk")
                        nc.vector.tensor_scalar(out=t_[:, :nn], in0=sl,
                                                scalar1=0.1, scalar2=None,
                                                op0=AL.mult)
                        nc.vector.tensor_tensor(out=sl, in0=sl,
                                                in1=t_[:, :nn], op=AL.max)
                    o_all = sp.tile([D, nb], F32, tag=f"o{side}")
                    for n0 in range(0, nb, 512):
                        nn = min(512, nb - n0)
                        op_ = psp.tile([D, 512], F32, tag="op", space="PSUM")
                        nc.tensor.matmul(out=op_[:, :nn], lhsT=w2[:],
                                         rhs=h_all[:, n0:n0 + nn],
                                         start=True, stop=True)
                        sl = o_all[:, n0:n0 + nn]
                        nc.vector.tensor_scalar(out=sl, in0=op_[:, :nn],
                                                scalar1=b2[:, 0:1],
                                                scalar2=None, op0=AL.add)
                        t_ = wk.tile([D, 512], F32, tag="lk2", name="lk2")
                        nc.vector.tensor_scalar(out=t_[:, :nn], in0=sl,
                                                scalar1=0.1, scalar2=None,
                                                op0=AL.mult)
                        nc.vector.tensor_tensor(out=sl, in0=sl,
                                                in1=t_[:, :nn], op=AL.max)
                    outs.append(o_all)

                prod = sp.tile([D, nb], F32, tag='prod', name='prod')
                nc.vector.tensor_tensor(out=prod[:], in0=outs[0][:],
                                        in1=outs[1][:], op=AL.mult)
                pred = sp.tile([1, nb], F32, tag='pred', name='pred')
                for n0 in range(0, nb, 512):
                    nn = min(512, nb - n0)
                    pp = psp.tile([1, 512], F32, tag="pp", space="PSUM")
                    nc.tensor.matmul(out=pp[:1, :nn], lhsT=on[:],
                                     rhs=prod[:, n0:n0 + nn],
                                     start=True, stop=True)
                    nc.vector.tensor_copy(out=pred[:, n0:n0 + nn],
                                          in_=pp[:1, :nn])
                nc.vector.tensor_tensor(out=pred[:], in0=pred[:], in1=bs[:],
                                        op=AL.add)
                nc.vector.tensor_tensor(out=pred[:], in0=pred[:], in1=rt[:],
                                        op=AL.subtract)
                nc.vector.tensor_tensor(out=pred[:], in0=pred[:], in1=pred[:],
                                        op=AL.mult)
                sse = sp.tile([1, 1], F32, tag='sse', name='sse')
                nc.vector.tensor_reduce(out=sse[:], in_=pred[:],
                                        axis=mybir.AxisListType.X, op=AL.add)
                nc.sync.dma_start(out=out[:], in_=sse[:])
    nc.finalize()
    return nc


# ------------------------------------------------------------ orchestration --

def _to_storage(arr, mp, width=None):
    w = arr.shape[1] if arr.ndim > 1 else 1
    out = np.zeros((mp['n_pad'], width or w), arr.dtype)
    out[mp['storage'], :w] = arr.reshape(len(arr), w)
    return out


def _shard(full, mp, core, C_pad, dtype):
    """storage-flat [n_pad, w] -> per-core [128, C_pad, w] (pad cols zero)."""
    C = mp['C']
    blk = full[core * mp['rows_per_core']:(core + 1) * mp['rows_per_core']]
    w = blk.shape[1]
    out = np.zeros((128, C_pad, w), dtype)
    out[:, :C, :] = blk.reshape(128, C, w)
    return np.ascontiguousarray(out)


def _unshard(results, key, mp, C_pad):
    """per-core [128, C_pad, w] -> storage-flat [n_pad, w] float32."""
    C = mp['C']
    outs = []
    for r in results:
        a = np.asarray(r[key])[:, :C, :].astype(np.float32)
        outs.append(a.reshape(mp['rows_per_core'], -1))
    return np.concatenate(outs, axis=0)


def _run(nc, in_maps, label):
    import time
    t0 = time.time()
    res = run_bass_kernel_spmd(nc, in_maps, core_ids=list(range(len(in_maps))))
    wall = time.time() - t0
    _EXEC_NS.setdefault("walls", []).append((label, wall))
    _EXEC_NS.setdefault("launches", []).append((label, nc, in_maps))
    return res.results


def kernel(ui_rows, ui_cols, ui_vals, iu_vals, d_i, d_j,
           embed_user, embed_item, add_w, fw1, fb1, fw2, fb2,
           user_bias, item_bias, avg_rating, user0, item_i0, ratings):
    ui_rows = np.asarray(ui_rows)
    ui_cols = np.asarray(ui_cols)
    mu = _side_mapping(U)
    mi = _side_mapping(I)
    w = np.asarray(add_w, np.float32)[0]

    planA = _plan_v2(mu['core'][ui_rows], mu['local'][ui_rows],
                     mi['storage'][ui_cols], np.asarray(ui_vals, np.float32),
                     mu['C'], mi['n_pad'], rcols=16)
    planB1 = _plan_v2(mi['core'][ui_cols], mi['local'][ui_cols],
                      mu['storage'][ui_rows], np.asarray(iu_vals, np.float32),
                      mi['C'], mu['n_pad'], rcols=48)
    planB2 = _plan_v2(mi['core'][ui_cols], mi['local'][ui_cols],
                      mu['storage'][ui_rows], np.asarray(iu_vals, np.float32),
                      mi['C'], mu['n_pad'], rcols=24)

    ncB1 = _build_side_v2(planB1, mu['n_pad'], 's1')
    ncA12 = _build_side_v2(planA, mi['n_pad'], 'dual', w1=float(w[1]),
                           w2=float(w[2]))
    ncB2 = _build_side_v2(planB2, mu['n_pad'], 's2', w2=float(w[2]))

    ue = np.asarray(embed_user, np.float32)
    ie = np.asarray(embed_item, np.float32)
    d_i = np.asarray(d_i, np.float32)
    d_j = np.asarray(d_j, np.float32)

    eu_st = _to_storage(ue, mu)                       # [n_pad,64] f32
    # B1: src=ue f32; rd = ie*d_j bf16 (item side)
    rdB = (ie * d_j[:, None]).astype(BF)
    rdB_st = _to_storage(rdB, mi)
    mapsB1 = []
    for c in range(NCORES):
        mapsB1.append({
            "src": eu_st,
            "idx": planB1['idx'][c], "pos": planB1['pos'][c],
            "val": planB1['val'][c],
            "rd": _shard(rdB_st, mi, c, planB1['C_pad'], BF),
        })
    rB1 = _run(ncB1, mapsB1, "B1")
    g1i = _unshard(rB1, "g1_out", mi, planB1['C_pad'])  # storage-flat f32

    # A12: src=[ie|g1_i] bf16; rd=ue*d_i; w0ue; dvec=d_i
    srcA = np.zeros((mi['n_pad'], 128), BF)
    srcA[:, :64] = _to_storage(ie.astype(BF), mi)
    srcA[:, 64:] = g1i.astype(BF)
    rdA_st = _to_storage((ue * d_i[:, None]).astype(BF), mu)
    w0ue_st = _to_storage((w[0] * ue).astype(BF), mu)
    di_st = _to_storage(d_i.astype(np.float32), mu)
    mapsA = []
    for c in range(NCORES):
        mapsA.append({
            "src": srcA,
            "idx": planA['idx'][c], "pos": planA['pos'][c],
            "val": planA['val'][c],
            "rd": _shard(rdA_st, mu, c, planA['C_pad'], BF),
            "w0ue": _shard(w0ue_st, mu, c, planA['C_pad'], BF),
            "dvec": np.ascontiguousarray(_shard(di_st, mu, c,
                           planA['C_pad'], np.float32)[:, :, 0]),
        })
    rA = _run(ncA12, mapsA, "A12")
    g1u = _unshard(rA, "g1_out", mu, planA['C_pad'])
    gcnu = _unshard(rA, "gcn_out", mu, planA['C_pad'])
    ssq_u = sum(float(r["stats"][0, 0]) for r in rA)

    # B2: src=[g1_u|0] bf16; rd = g1_i*d_j; wcomb = w0*ie + w1*g1_i
    srcB2 = np.zeros((mu['n_pad'], 128), BF)
    srcB2[:, :64] = g1u.astype(BF)
    rd2_st = np.zeros((mi['n_pad'], 64), BF)
    rd2_st[:] = (g1i * _to_storage(d_j.reshape(-1, 1), mi)).astype(BF)
    wcomb_st = (w[0] * _to_storage(ie, mi) + w[1] * g1i).astype(BF)
    mapsB2 = []
    for c in range(NCORES):
        mapsB2.append({
            "src": srcB2,
            "idx": planB2['idx'][c], "pos": planB2['pos'][c],
            "val": planB2['val'][c],
            "rd": _shard(rd2_st, mi, c, planB2['C_pad'], BF),
            "wcomb": _shard(wcomb_st, mi, c, planB2['C_pad'], BF),
        })
    rB2 = _run(ncB2, mapsB2, "B2")
    gcni = _unshard(rB2, "gcn_out", mi, planB2['C_pad'])
    ssq_i = sum(float(r["stats"][0, 0]) for r in rB2)

    # head
    nb = B // NCORES
    user0 = np.asarray(user0)
    item_i0 = np.asarray(item_i0)
    xu_rows = gcnu[mu['storage'][user0]]
    xi_rows = gcni[mi['storage'][item_i0]]
    bsum = (np.asarray(user_bias, np.float32)[user0, 0]
            + np.asarray(item_bias, np.float32)[item_i0, 0]
            + np.float32(np.asarray(avg_rating, np.float32)[0]))
    nch = _build_head(nb)
    _EXEC_NS['headnb'] = nb
    hmaps = []
    for c in range(NCORES):
        sl = slice(c * nb, (c + 1) * nb)
        hmaps.append({
            "xu": np.ascontiguousarray(xu_rows[sl].T),
            "xi": np.ascontiguousarray(xi_rows[sl].T),
            "fw1t": np.ascontiguousarray(np.asarray(fw1, np.float32).T),
            "fb1": np.asarray(fb1, np.float32).reshape(2 * D, 1),
            "fw2t": np.ascontiguousarray(np.asarray(fw2, np.float32).T),
            "fb2": np.asarray(fb2, np.float32).reshape(D, 1),
            "bsum": bsum[sl].reshape(1, nb),
            "rat": np.asarray(ratings, np.float32)[sl].reshape(1, nb),
        })
    rH = _run(nch, hmaps, "H")
    sse = sum(float(r["out"][0, 0]) for r in rH)

    loss = (sse / B + LAM * ssq_u / (U * D) + LAM * ssq_i / (I * D))
    return np.float32(loss)
